# revision 2
# baseline (speedup 1.0000x reference)
"""Trainium2 Bass kernel for nn_CosineSimilarityLayer (dual-ring serial v4).

out = l2norm_rows(x) @ l2norm_rows_over_N(W)       x:[4096,512]  W:[512,5994]

Math:  out[b,n] = sum_d (x[b,d]*xscale[b]) * wscale[d] * W[d,n]
  xscale[b] = rsqrt(sum_d x[b,d]^2)   (folded into x rows pre-transpose)
  wscale[d] = rsqrt(sum_n W[d,n]^2)   (folded into transposed x per dt)

Sharding: data-parallel over batch - 8 cores x [512, 512] x-shards, W
replicated.  No collectives.

v4 vs the 102.5us serial v2 (measured facts from v2/v3 traces):
  * W streams on BOTH HWDGE rings (Sync + Activation, 16 queues each) as
    24 slabs - the single ring capped at ~400 GB/s.  W lands straight in
    the resident f32r tile (f32r ExternalInput; no staging, no cast
    pass, which cost DVE 15.6us in v2).
  * x as 4 per-bt DMAs issued first on the Act ring (a single 1MB DMA
    sat on one queue for 12us in v3).
  * W squares split DVE/ACT (one engine alone lags the stream by ~9us,
    which was v2's 10us post-stream bubble); per-dt reduce+sqrt+recip
    chains close ~1us after each dt's tapered last slab.
  * PSUM evictions (plain copies - both scales pre-folded) alternate
    DVE/ACT; PSUM-sourced ops cost ~1.3ns/col so one engine alone adds
    lag.  Output DMAs alternate rings per 1024-col pair.
  * Matmuls stay serial 4-dt PSUM accumulation (measured: any extra
    eviction pass costs ~32us of engine time - a mid-stream matmul
    start is not worth 2x evictions).
"""

import os
import sys
import types
from contextlib import ExitStack

import numpy as np


def _ensure_axon_hooks():
    try:
        import antenv.axon_hooks  # noqa: F401
        return
    except ImportError:
        pass
    try:
        import antenv
    except ImportError:
        return
    m = types.ModuleType("antenv.axon_hooks")
    holder = {"h": None}
    m.set_axon_ntff_profile_hook = lambda h: holder.__setitem__("h", h)
    m.get_axon_ntff_profile_hook = lambda: holder["h"]
    sys.modules["antenv.axon_hooks"] = m
    antenv.axon_hooks = m
    try:
        from trn_agent_boot.trn_boot import _ntff_profile_via_ctypes
        so = "/opt/axon/libaxon_pjrt.so"
        if os.path.exists(so):
            m.set_axon_ntff_profile_hook(_ntff_profile_via_ctypes(so))
    except Exception:
        pass


_ensure_axon_hooks()

import concourse.bass as bass
import concourse.tile as tile
from concourse import bacc, mybir
from concourse.bass_utils import run_bass_kernel_spmd
from concourse.masks import make_identity

F32 = mybir.dt.float32
F32R = mybir.dt.float32r
AF = mybir.ActivationFunctionType
MUL = mybir.AluOpType.mult
AXX = mybir.AxisListType.X

B, D, N = 4096, 512, 5994
NCORES = 8
P = 128
BSH = B // NCORES          # 512 rows of x per core
BT = BSH // P              # 4 b-tiles
DT = D // P                # 4 d-tiles (contraction)
CHUNK = 512                # output n-chunk (one PSUM bank of fp32)
# W stream plan.  Measured DMA behavior: ~8 DMAs in flight (recycled
# semaphore pool), ~55GB/s per queue, and fewer/bigger slabs reach the
# ~430GB/s aggregate cap while many small ones only hit ~370.  So: big
# 1499-col slabs for the bulk, with dt3's big slabs FIRST (their squares
# finish mid-stream) and small tapered tails for dt2+dt3 at the end, so
# only two tiny squares + the dt2/dt3 chains trail the stream end.
# STREAM entries: (dt, col_offset, width) in issue order.
STREAM = (
    [(3, o, 1024) for o in range(0, 5120, 1024)] +
    [(0, o, 1024) for o in range(0, 5120, 1024)] + [(0, 5120, 874)] +
    [(1, o, 1024) for o in range(0, 5120, 1024)] + [(1, 5120, 874)] +
    [(2, o, 1024) for o in range(0, 5120, 1024)] +
    [(2, 5120, 300), (2, 5420, 300), (2, 5720, 274)] +
    [(3, 5120, 300), (3, 5420, 250), (3, 5670, 200), (3, 5870, 124)]
)
_nsl = {dt: sum(1 for e in STREAM if e[0] == dt) for dt in range(DT)}
assert all(sum(w for d, _, w in STREAM if d == dt) == N for dt in range(DT))
MAXSLAB = max(_nsl.values())

CHUNKS = []
_n0 = 0
while _n0 < N:
    CHUNKS.append((_n0, min(CHUNK, N - _n0)))
    _n0 += CHUNK
NCH = len(CHUNKS)          # 12

PAIRS = [(bt, ch) for bt in range(BT) for ch in range(NCH)]


def _build():
    nc = bacc.Bacc("TRN2", target_bir_lowering=False, debug=False,
                   num_devices=NCORES)

    x_d = nc.dram_tensor("x", [BSH, D], F32, kind="ExternalInput").ap()
    w_d = nc.dram_tensor("W", [D, N], F32R, kind="ExternalInput").ap()
    o_d = nc.dram_tensor("out", [BSH, N], F32, kind="ExternalOutput").ap()

    x_r = x_d.rearrange("(t p) d -> p t d", p=P)        # [128, 4, 512]
    w_r = w_d.rearrange("(t p) n -> p t n", p=P)        # [128, 4, 5994]
    o_r = o_d.rearrange("(t p) n -> p t n", p=P)        # [128, 4, 5994]

    with tile.TileContext(nc) as tc, ExitStack() as ctx:
        const = ctx.enter_context(tc.tile_pool(name="const", bufs=1))
        xp = ctx.enter_context(tc.tile_pool(name="xp", bufs=1))
        sc = ctx.enter_context(tc.tile_pool(name="sc", bufs=1))
        xt = ctx.enter_context(tc.tile_pool(name="xt", bufs=1))
        wp = ctx.enter_context(tc.tile_pool(name="wp", bufs=1))
        trv = ctx.enter_context(tc.tile_pool(name="trv", bufs=2))
        tra = ctx.enter_context(tc.tile_pool(name="tra", bufs=2))
        trg = ctx.enter_context(tc.tile_pool(name="trg", bufs=1))
        ostp = ctx.enter_context(tc.tile_pool(name="ostp", bufs=4))
        tp = ctx.enter_context(tc.tile_pool(name="tp", bufs=4, space="PSUM"))
        mm = ctx.enter_context(tc.tile_pool(name="mm", bufs=4, space="PSUM"))

        identity = const.tile([P, P], F32)
        make_identity(nc, identity)

        wr1 = wp.tile([P, DT, N], F32R)          # resident W (f32r bits)
        x_sb = xp.tile([P, BT, D], F32)          # x shard (scaled in place)
        xtr1 = xt.tile([P, DT, BSH], F32R)       # ws-folded x^T
        wsqp = sc.tile([P, DT, MAXSLAB], F32)    # per-slab square sums
        xsq = sc.tile([P, BT], F32)
        xsr = sc.tile([P, BT], F32)
        xsc = sc.tile([P, BT], F32)
        wsq = sc.tile([P, DT, 1], F32)
        wsr = sc.tile([P, DT, 1], F32)
        wsc = sc.tile([P, DT, 1], F32)

        # --- input DMAs.  x first on the Act ring (4 queue-parallel DMAs -
        # a single 1MB DMA sat on one ~55GB/s queue for 12us), W entirely on
        # the Sync queue: DMA issues wait on a recycled 8-semaphore pool, so
        # any W issue on the Act ring blocks ALL later ACT compute (v4
        # measured ACT's first square at t=35us because of this).
        for bt in range(BT):
            nc.scalar.dma_start(x_sb[:, bt, :], x_r[:, bt, :])
        # preload the ACT Sqrt/Square function table now (lazy load costs
        # 1.3us on the critical chain otherwise)
        warm = sc.tile([P, 1], F32)
        nc.scalar.sqrt(warm, identity[:, 0:1])
        for (dt, s0, sw) in STREAM:
            nc.sync.dma_start(wr1[:, dt, s0:s0 + sw],
                              w_r[:, dt, s0:s0 + sw])

        # --- square helpers (accum_out -> wsqp slot)
        def sq_dve(dt, s0, sw, slot):
            sl = wr1[:, dt, s0:s0 + sw]
            t = trv.tile([P, 1499], F32, tag="trv")
            nc.vector.scalar_tensor_tensor(
                out=t[:, :sw], in0=sl, scalar=1.0, in1=sl,
                op0=MUL, op1=MUL, accum_out=wsqp[:, dt, slot:slot + 1])

        def sq_act(dt, s0, sw, slot):
            sl = wr1[:, dt, s0:s0 + sw]
            t = tra.tile([P, 1499], F32, tag="tra")
            nc.scalar.activation(t[:, :sw], sl, AF.Square,
                                 accum_out=wsqp[:, dt, slot:slot + 1])

        # --- x chain: sumsq on DVE (stt+accum, x lands first), sqrt+scale
        # on ACT, recip on DVE.
        for bt in range(BT):
            tg = trg.tile([P, D], F32, tag="trg")
            nc.vector.scalar_tensor_tensor(
                out=tg, in0=x_sb[:, bt, :], scalar=1.0, in1=x_sb[:, bt, :],
                op0=MUL, op1=MUL, accum_out=xsq[:, bt:bt + 1])
        nc.scalar.sqrt(xsr, xsq)
        nc.vector.reciprocal(xsc, xsr)
        for bt in range(BT):
            nc.scalar.activation(x_sb[:, bt, :], x_sb[:, bt, :], AF.Copy,
                                 scale=xsc[:, bt:bt + 1])
        # transposes into 4 PSUM slots while PE is otherwise idle
        pts = {}
        for dt in range(DT):
            pt = tp.tile([P, BSH], F32, tag="pt")
            pts[dt] = pt
            for bt in range(BT):
                nc.tensor.transpose(pt[:, bt * P:(bt + 1) * P],
                                    x_sb[:, bt, dt * P:(dt + 1) * P],
                                    identity)

        # --- squares in stream (arrival) order, alternating DVE/ACT per
        # slab (one engine alone can't keep up with ~430GB/s of W).  Each
        # dt's chain (reduce DVE -> sqrt ACT -> recip DVE -> one-op fold,
        # alternating engines) is emitted right after that dt's last slab;
        # only dt2/dt3's tiny tail squares + chains trail the stream end.
        def chain_dt(dt, fold_act):
            nc.vector.reduce_sum(wsq[:, dt, :], wsqp[:, dt, :_nsl[dt]],
                                 axis=AXX)
            nc.scalar.sqrt(wsr[:, dt, :], wsq[:, dt, :])
            nc.vector.reciprocal(wsc[:, dt, :], wsr[:, dt, :])
            if dt >= 2:
                # end-of-stream dts: per-bt folds, bt0 first - the matmul
                # loop (bt-major) only needs bt0's fold to start; the rest
                # hide behind bt0's ~11us of matmuls
                for bt in range(BT):
                    sl = (xtr1[:, dt, bt * P:(bt + 1) * P],
                          pts[dt][:, bt * P:(bt + 1) * P])
                    if fold_act:
                        nc.scalar.activation(sl[0], sl[1], AF.Copy,
                                             scale=wsc[:, dt, :])
                    else:
                        nc.vector.tensor_scalar_mul(sl[0], sl[1],
                                                    wsc[:, dt, :])
            elif fold_act:
                nc.scalar.activation(xtr1[:, dt, :], pts[dt], AF.Copy,
                                     scale=wsc[:, dt, :])
            else:
                nc.vector.tensor_scalar_mul(xtr1[:, dt, :], pts[dt],
                                            wsc[:, dt, :])

        slot = {dt: 0 for dt in range(DT)}
        done = {dt: 0 for dt in range(DT)}
        for k, (dt, s0, sw) in enumerate(STREAM):
            if k % 2 == 0:
                sq_dve(dt, s0, sw, slot[dt])
            else:
                sq_act(dt, s0, sw, slot[dt])
            slot[dt] += 1
            done[dt] += sw
            if done[dt] == N:
                chain_dt(dt, fold_act=(dt % 2 == 1))

        # --- serial matmul: per (bt, ch) 4-dt PSUM accumulation, eviction
        # (plain copy, scales pre-folded) alternating DVE/ACT, out DMA per
        # 1024-col pair alternating rings.  A queue moves ~55GB/s, so the
        # final transfers are split per-chunk across both rings to shrink
        # the completion tail.
        ost = None
        k = 0
        npair = len(PAIRS) // 2
        for (bt, ch) in PAIRS:
            n0, nw = CHUNKS[ch]
            ps = mm.tile([P, CHUNK], F32, tag="mm")
            for dt in range(DT):
                nc.tensor.matmul(ps[:, :nw],
                                 xtr1[:, dt, bt * P:(bt + 1) * P],
                                 wr1[:, dt, n0:n0 + nw],
                                 start=(dt == 0), stop=(dt == DT - 1))
            if ch % 2 == 0:
                ost = ostp.tile([P, 2 * CHUNK], F32, tag="ost")
                nc.vector.tensor_copy(ost[:, :nw], ps[:, :nw])
            else:
                nc.scalar.activation(ost[:, CHUNK:CHUNK + nw], ps[:, :nw],
                                     AF.Copy)
                gn0, _ = CHUNKS[ch - 1]
                if k >= npair - 2:
                    # tail pairs: small DMAs across both rings so the final
                    # transfer is short on a ~55GB/s queue (128-col = 64KB
                    # for the very last pair)
                    step = 128 if k == npair - 1 else 256
                    q = 0
                    while q < CHUNK + nw:
                        qw = min(step, CHUNK + nw - q)
                        eng = nc.sync if (q // step) % 2 == 0 else nc.scalar
                        eng.dma_start(o_r[:, bt, gn0 + q:gn0 + q + qw],
                                      ost[:, q:q + qw])
                        q += qw
                else:
                    eng = nc.sync if k % 2 == 0 else nc.scalar
                    eng.dma_start(o_r[:, bt, gn0:gn0 + CHUNK + nw],
                                  ost[:, :CHUNK + nw])
                k += 1

    nc.compile()
    return nc


LAST_RESULT = None


def kernel(x: np.ndarray, W: np.ndarray) -> np.ndarray:
    global LAST_RESULT
    x = np.ascontiguousarray(x, dtype=np.float32)
    W = np.ascontiguousarray(W, dtype=np.float32)
    assert x.shape == (B, D) and W.shape == (D, N)

    nc = _build()

    in_maps = [{"x": np.ascontiguousarray(x[c * BSH:(c + 1) * BSH]), "W": W}
               for c in range(NCORES)]

    res = run_bass_kernel_spmd(nc, in_maps, core_ids=list(range(NCORES)))
    LAST_RESULT = res
    return np.concatenate([res.results[c]["out"] for c in range(NCORES)],
                          axis=0)


# revision 3
# speedup vs baseline: 1.1099x; 1.1099x over previous
"""Trainium2 Bass kernel for nn_CosineSimilarityLayer (serial v13).

out = l2norm_rows(x) @ l2norm_rows_over_N(W)       x:[4096,512]  W:[512,5994]

Math:  out[b,n] = xscale[b] * sum_d x[b,d] * wscale[d] * W[d,n]
  wscale[d] = rsqrt(sum_n W[d,n]^2)  - folded into the transposed x, per dt
  xscale[b] = rsqrt(sum_d x[b,d]^2)  - folded into the PSUM evictions
  (where partitions = b), so raw x feeds the PE transposes immediately.
  (the reference's max(.,eps) clamp is a numerical no-op for these inputs)

Sharding: data-parallel over batch - 8 cores x [512, 512] x-shards, W
replicated.  No collectives (a tiny 8-core AllReduce measures ~55us on
this fleet); wscale needs all of W, so the matmuls are gated on the full
W stream either way.

Measured hardware facts this schedule is built on (v2-v12 traces):
  * 16 shared DMA engines, ~400GB/s aggregate, ~8 DMAs in flight
    (recycled semaphore pool) - per-instruction rate is aggregate/
    n_inflight, so the stream is wave-quantized and its TAIL costs the
    full solo-transfer time of the last slabs.  Hence: 1024-col W slabs
    on the Sync queue only (DMA issues block the issuing engine's queue
    on semaphore reuse - issuing W on the Act ring stalls ACT compute
    until end of stream), dt3 last, tapering to 96 cols.
  * W lands directly in the resident f32r tile (f32r ExternalInput,
    same bits as f32) - no staging, no 15.6us DVE cast pass.
  * x as 4 per-bt DMAs on the Act ring first (a single 1MB DMA sits on
    one slice of the engine pool for ~12us).
  * One engine cannot square W at stream pace: squares alternate
    DVE(stt+accum)/ACT(Square+accum) per slab in stream order; each
    dt's reduce->sqrt->recip->fold chain closes in the next dt's
    shadow.  dt2/dt3 end in small slabs so only tiny squares + chains
    trail the stream end; dt3's folds are per-bt (bt0 first) and the
    matmul loop is bt-major, so PE starts ~0.7us before fold3 lands.
  * Matmuls: serial per-(bt,chunk) 4-dt PSUM accumulation, 4 banks
    rotating (+4 banks hold the x^T transposes until their folds).
    PSUM-sourced ops cost ~1.3ns/col, so an extra eviction pass (any
    mid-stream matmul overlap scheme) costs ~32us of DVE/ACT time and
    nets zero after SBUF-contention slows the stream (measured, v12).
  * Evictions fold xscale (DVE tensor_scalar / ACT Copy+scale
    alternating per chunk); output DMAs alternate rings per 1024-col
    pair; the last two pairs ship as ring-parallel halves since a lone
    tail DMA owns most of the engine pool.

Measured (noisy fleet, +-4us run to run): ~95-99us vs 102.5-106us for
the v2 baseline; rel err 1.55e-4.
"""

import os
import sys
import types
from contextlib import ExitStack

import numpy as np


def _ensure_axon_hooks():
    try:
        import antenv.axon_hooks  # noqa: F401
        return
    except ImportError:
        pass
    try:
        import antenv
    except ImportError:
        return
    m = types.ModuleType("antenv.axon_hooks")
    holder = {"h": None}
    m.set_axon_ntff_profile_hook = lambda h: holder.__setitem__("h", h)
    m.get_axon_ntff_profile_hook = lambda: holder["h"]
    sys.modules["antenv.axon_hooks"] = m
    antenv.axon_hooks = m
    try:
        from trn_agent_boot.trn_boot import _ntff_profile_via_ctypes
        so = "/opt/axon/libaxon_pjrt.so"
        if os.path.exists(so):
            m.set_axon_ntff_profile_hook(_ntff_profile_via_ctypes(so))
    except Exception:
        pass


_ensure_axon_hooks()

import concourse.bass as bass
import concourse.tile as tile
from concourse import bacc, mybir
from concourse.bass_utils import run_bass_kernel_spmd
from concourse.masks import make_identity

F32 = mybir.dt.float32
F32R = mybir.dt.float32r
AF = mybir.ActivationFunctionType
MUL = mybir.AluOpType.mult
AXX = mybir.AxisListType.X

B, D, N = 4096, 512, 5994
NCORES = 8
P = 128
BSH = B // NCORES          # 512 rows of x per core
BT = BSH // P              # 4 b-tiles
DT = D // P                # 4 d-tiles (contraction)
CHUNK = 512                # output n-chunk (one PSUM bank of fp32)
# W stream plan.  Measured DMA behavior: ~8 DMAs in flight (recycled
# semaphore pool), ~55GB/s per queue, and fewer/bigger slabs reach the
# ~430GB/s aggregate cap while many small ones only hit ~370.  So: big
# 1499-col slabs for the bulk, with dt3's big slabs FIRST (their squares
# finish mid-stream) and small tapered tails for dt2+dt3 at the end, so
# only two tiny squares + the dt2/dt3 chains trail the stream end.
# STREAM entries: (dt, col_offset, width) in issue order.
STREAM = (
    [(3, o, 1024) for o in range(0, 5120, 1024)] +
    [(0, o, 1024) for o in range(0, 5120, 1024)] + [(0, 5120, 874)] +
    [(1, o, 1024) for o in range(0, 5120, 1024)] + [(1, 5120, 874)] +
    [(2, o, 1024) for o in range(0, 5120, 1024)] +
    [(2, 5120, 300), (2, 5420, 300), (2, 5720, 274)] +
    [(3, 5120, 300), (3, 5420, 250), (3, 5670, 228), (3, 5898, 96)]
)
_nsl = {dt: sum(1 for e in STREAM if e[0] == dt) for dt in range(DT)}
assert all(sum(w for d, _, w in STREAM if d == dt) == N for dt in range(DT))
MAXSLAB = max(_nsl.values())

CHUNKS = []
_n0 = 0
while _n0 < N:
    CHUNKS.append((_n0, min(CHUNK, N - _n0)))
    _n0 += CHUNK
NCH = len(CHUNKS)          # 12

PAIRS = [(bt, ch) for bt in range(BT) for ch in range(NCH)]


def _build():
    nc = bacc.Bacc("TRN2", target_bir_lowering=False, debug=False,
                   num_devices=NCORES)

    x_d = nc.dram_tensor("x", [BSH, D], F32, kind="ExternalInput").ap()
    w_d = nc.dram_tensor("W", [D, N], F32R, kind="ExternalInput").ap()
    o_d = nc.dram_tensor("out", [BSH, N], F32, kind="ExternalOutput").ap()

    x_r = x_d.rearrange("(t p) d -> p t d", p=P)        # [128, 4, 512]
    w_r = w_d.rearrange("(t p) n -> p t n", p=P)        # [128, 4, 5994]
    o_r = o_d.rearrange("(t p) n -> p t n", p=P)        # [128, 4, 5994]

    with tile.TileContext(nc) as tc, ExitStack() as ctx:
        const = ctx.enter_context(tc.tile_pool(name="const", bufs=1))
        xp = ctx.enter_context(tc.tile_pool(name="xp", bufs=1))
        sc = ctx.enter_context(tc.tile_pool(name="sc", bufs=1))
        xt = ctx.enter_context(tc.tile_pool(name="xt", bufs=1))
        wp = ctx.enter_context(tc.tile_pool(name="wp", bufs=1))
        trv = ctx.enter_context(tc.tile_pool(name="trv", bufs=2))
        tra = ctx.enter_context(tc.tile_pool(name="tra", bufs=2))
        trg = ctx.enter_context(tc.tile_pool(name="trg", bufs=1))
        ostp = ctx.enter_context(tc.tile_pool(name="ostp", bufs=4))
        tp = ctx.enter_context(tc.tile_pool(name="tp", bufs=4, space="PSUM"))
        mm = ctx.enter_context(tc.tile_pool(name="mm", bufs=4, space="PSUM"))

        identity = const.tile([P, P], F32)
        make_identity(nc, identity)

        wr1 = wp.tile([P, DT, N], F32R)          # resident W (f32r bits)
        x_sb = xp.tile([P, BT, D], F32)          # x shard (scaled in place)
        xtr1 = xt.tile([P, DT, BSH], F32R)       # ws-folded x^T
        wsqp = sc.tile([P, DT, MAXSLAB], F32)    # per-slab square sums
        xsq = sc.tile([P, BT], F32)
        xsr = sc.tile([P, BT], F32)
        xsc = sc.tile([P, BT], F32)
        wsq = sc.tile([P, DT, 1], F32)
        wsr = sc.tile([P, DT, 1], F32)
        wsc = sc.tile([P, DT, 1], F32)

        # --- input DMAs.  x first on the Act ring (4 queue-parallel DMAs -
        # a single 1MB DMA sat on one ~55GB/s queue for 12us), W entirely on
        # the Sync queue: DMA issues wait on a recycled 8-semaphore pool, so
        # any W issue on the Act ring blocks ALL later ACT compute (v4
        # measured ACT's first square at t=35us because of this).
        for bt in range(BT):
            nc.scalar.dma_start(x_sb[:, bt, :], x_r[:, bt, :])
        # preload the ACT Sqrt/Square function table now (lazy load costs
        # 1.3us on the critical chain otherwise)
        warm = sc.tile([P, 1], F32)
        nc.scalar.sqrt(warm, identity[:, 0:1])
        for (dt, s0, sw) in STREAM:
            nc.sync.dma_start(wr1[:, dt, s0:s0 + sw],
                              w_r[:, dt, s0:s0 + sw])

        # --- square helpers (accum_out -> wsqp slot)
        def sq_dve(dt, s0, sw, slot):
            sl = wr1[:, dt, s0:s0 + sw]
            t = trv.tile([P, 1499], F32, tag="trv")
            nc.vector.scalar_tensor_tensor(
                out=t[:, :sw], in0=sl, scalar=1.0, in1=sl,
                op0=MUL, op1=MUL, accum_out=wsqp[:, dt, slot:slot + 1])

        def sq_act(dt, s0, sw, slot):
            sl = wr1[:, dt, s0:s0 + sw]
            t = tra.tile([P, 1499], F32, tag="tra")
            nc.scalar.activation(t[:, :sw], sl, AF.Square,
                                 accum_out=wsqp[:, dt, slot:slot + 1])

        # --- x chain: sumsq on DVE (stt+accum, x lands first), sqrt ACT,
        # recip DVE.  xscale is folded into the PSUM evictions (where the
        # partition dim is b), so raw x feeds the transposes immediately.
        for bt in range(BT):
            tg = trg.tile([P, D], F32, tag="trg")
            nc.vector.scalar_tensor_tensor(
                out=tg, in0=x_sb[:, bt, :], scalar=1.0, in1=x_sb[:, bt, :],
                op0=MUL, op1=MUL, accum_out=xsq[:, bt:bt + 1])
        nc.scalar.sqrt(xsr, xsq)
        nc.vector.reciprocal(xsc, xsr)
        # transposes of raw x into 4 PSUM slots while PE is otherwise idle
        pts = {}
        for dt in range(DT):
            pt = tp.tile([P, BSH], F32, tag="pt")
            pts[dt] = pt
            for bt in range(BT):
                nc.tensor.transpose(pt[:, bt * P:(bt + 1) * P],
                                    x_sb[:, bt, dt * P:(dt + 1) * P],
                                    identity)

        # --- squares in stream (arrival) order, alternating DVE/ACT per
        # slab (one engine alone can't keep up with ~430GB/s of W).  Each
        # dt's chain (reduce DVE -> sqrt ACT -> recip DVE -> one-op fold,
        # alternating engines) is emitted right after that dt's last slab;
        # only dt2/dt3's tiny tail squares + chains trail the stream end.
        def chain_dt(dt, fold_act):
            nc.vector.reduce_sum(wsq[:, dt, :], wsqp[:, dt, :_nsl[dt]],
                                 axis=AXX)
            nc.scalar.sqrt(wsr[:, dt, :], wsq[:, dt, :])
            nc.vector.reciprocal(wsc[:, dt, :], wsr[:, dt, :])
            if dt >= 2:
                # end-of-stream dts: per-bt folds, bt0 first - the matmul
                # loop (bt-major) only needs bt0's fold to start; the rest
                # hide behind bt0's ~11us of matmuls
                for bt in range(BT):
                    sl = (xtr1[:, dt, bt * P:(bt + 1) * P],
                          pts[dt][:, bt * P:(bt + 1) * P])
                    if fold_act:
                        nc.scalar.activation(sl[0], sl[1], AF.Copy,
                                             scale=wsc[:, dt, :])
                    else:
                        nc.vector.tensor_scalar_mul(sl[0], sl[1],
                                                    wsc[:, dt, :])
            elif fold_act:
                nc.scalar.activation(xtr1[:, dt, :], pts[dt], AF.Copy,
                                     scale=wsc[:, dt, :])
            else:
                nc.vector.tensor_scalar_mul(xtr1[:, dt, :], pts[dt],
                                            wsc[:, dt, :])

        slot = {dt: 0 for dt in range(DT)}
        done = {dt: 0 for dt in range(DT)}
        for k, (dt, s0, sw) in enumerate(STREAM):
            if k % 2 == 0:
                sq_dve(dt, s0, sw, slot[dt])
            else:
                sq_act(dt, s0, sw, slot[dt])
            slot[dt] += 1
            done[dt] += sw
            if done[dt] == N:
                chain_dt(dt, fold_act=(dt % 2 == 1))

        # --- serial matmul: per (bt, ch) 4-dt PSUM accumulation, eviction
        # (plain copy, scales pre-folded) alternating DVE/ACT, out DMA per
        # 1024-col pair alternating rings.  A queue moves ~55GB/s, so the
        # final transfers are split per-chunk across both rings to shrink
        # the completion tail.
        ost = None
        k = 0
        npair = len(PAIRS) // 2
        for (bt, ch) in PAIRS:
            n0, nw = CHUNKS[ch]
            ps = mm.tile([P, CHUNK], F32, tag="mm")
            for dt in range(DT):
                nc.tensor.matmul(ps[:, :nw],
                                 xtr1[:, dt, bt * P:(bt + 1) * P],
                                 wr1[:, dt, n0:n0 + nw],
                                 start=(dt == 0), stop=(dt == DT - 1))
            if ch % 2 == 0:
                ost = ostp.tile([P, 2 * CHUNK], F32, tag="ost")
                nc.vector.tensor_scalar_mul(ost[:, :nw], ps[:, :nw],
                                            xsc[:, bt:bt + 1])
            else:
                nc.scalar.activation(ost[:, CHUNK:CHUNK + nw], ps[:, :nw],
                                     AF.Copy, scale=xsc[:, bt:bt + 1])
                gn0, _ = CHUNKS[ch - 1]
                if k >= npair - 2:
                    # tail pairs: one DMA per ring (a lone DMA gets a big
                    # share of the 16 shared engines, so halves beat many
                    # small issues whose 0.6us issue cost dominates)
                    half = (CHUNK + nw) // 2
                    nc.sync.dma_start(o_r[:, bt, gn0:gn0 + half],
                                      ost[:, :half])
                    nc.scalar.dma_start(
                        o_r[:, bt, gn0 + half:gn0 + CHUNK + nw],
                        ost[:, half:CHUNK + nw])
                else:
                    eng = nc.sync if k % 2 == 0 else nc.scalar
                    eng.dma_start(o_r[:, bt, gn0:gn0 + CHUNK + nw],
                                  ost[:, :CHUNK + nw])
                k += 1

    nc.compile()
    return nc


LAST_RESULT = None


def kernel(x: np.ndarray, W: np.ndarray) -> np.ndarray:
    global LAST_RESULT
    x = np.ascontiguousarray(x, dtype=np.float32)
    W = np.ascontiguousarray(W, dtype=np.float32)
    assert x.shape == (B, D) and W.shape == (D, N)

    nc = _build()

    in_maps = [{"x": np.ascontiguousarray(x[c * BSH:(c + 1) * BSH]), "W": W}
               for c in range(NCORES)]

    res = run_bass_kernel_spmd(nc, in_maps, core_ids=list(range(NCORES)))
    LAST_RESULT = res
    return np.concatenate([res.results[c]["out"] for c in range(NCORES)],
                          axis=0)
